# revision 20
# baseline (speedup 1.0000x reference)
"""Distributed GATv1 (2x GAT + SAGE + MLP head) for Trainium2, 8 NeuronCores.

Strategy (graph/data parallel per the sharding hint, restructured for speed):
- Nodes are sharded across 8 cores; local nodes are sorted by in-degree and
  packed into 49 bins of 128 (dst-major layout): partition p of bin b owns
  one destination, its in-edges live on the free axis. Degree sorting makes
  per-bin max degree tight (~2% edge padding).
- Layer 1 does NO device gather: the host ships edge-ordered x columns
  (xT[:, src(e)] per edge slot), so per-edge [h|al_s|al_d] rows come straight
  out of PE matmuls. No first AllGather either.
- Layers 2/3 gather per-edge rows with dma_gather (one call per bin+side,
  round-robined over 4 SWDGE queues - queue-level concurrency hides HBM
  random-read latency, ~3x over one queue). dma_gather indices are int16, so
  the 50176-row tables are split into two overlapping chunk-aligned halves
  (A=[0,28672), B=[21504,50176)); edges with src in the overlap are assigned
  to whichever side balances each bin's max count. Pad slots point at a pad
  row with al_src=-1000 so they contribute exp(-inf)=0 weight.
- w = exp(leakyrelu(al_s+al_d)) via two exps + max; messages scaled in
  place; one strided tensor_reduce per tile yields both message sums and
  softmax denominators. Self-loops are ordinary edges (src = own row, always
  slot 0 of their side) and also deliver al_dst for the tile.
- The next layer's dense projection is fused into each edge phase epilogue;
  AllGathers are issued in 7 chunks interleaved with the producing phase so
  they hide under compute. SAGE's linears commute with mean-aggregation:
  edge-2's epilogue computes y=f3@(Wl M1 M2) and r=f3@(Wr M1 M2) [16 cols],
  so SAGE only gathers tiny y rows and does out=sigmoid(mean_agg(y)+r+c).
"""

import numpy as np
import ml_dtypes

# Problem constants (hardcoded; kernel.py must be self-contained).
N = 50000
E = 800000
IN_C = 128
HID = 64
HEADS = 3
OUT_C = 16
C = HEADS * HID          # 192
COLS = 256               # g-table row width (512B bf16 rows)
YCOLS = 64               # y-table row width (256B f32 rows)
NCORES = 8
P = 128
NPC = N // NCORES        # 6250
NP = 6272                # padded local nodes (49*128)
NT = NP // P             # 49 bins
NCH = 7                  # allgather chunks
CR = NP // NCH           # 896 local rows per chunk
NG = NCORES * NP         # 50176 global rows
GCH = NCORES * CR        # 7168 global rows per chunk
ALO = 3 * GCH            # 21504: B table covers global rows [ALO, NG)
AHI = 4 * GCH            # 28672: A table covers [0, AHI); also pad row index
TSIZE = AHI + P          # table rows incl pad row (both tables same size)
SELF_A_BINS = 28         # bins 0..27 (chunks 0-3, gid<28672) self row in A
GRP = 2                  # bins per gather group in layers 2/3
BF16 = ml_dtypes.bfloat16


def _wrap16(flat):
    """[n] flat idx list (i = j*128+p order) -> [16, n/16] wrapped layout."""
    n = len(flat)
    out = np.empty((16, n // 16), np.int16)
    for p in range(16):
        out[p] = flat[p::16]
    return out


def preprocess(x, edge_index):
    src = np.asarray(edge_index[0], np.int64)
    dst = np.asarray(edge_index[1], np.int64)
    x = np.asarray(x, np.float32)
    deg = np.bincount(dst, minlength=N)

    gid_of = np.zeros(N, np.int64)
    lp_of = np.zeros(N, np.int64)
    gid_pos_all = []
    T1_k = np.zeros((NCORES, NT), np.int64)
    for k in range(NCORES):
        d_pad = np.zeros(NP, np.int64)
        d_pad[:NPC] = deg[k * NPC:(k + 1) * NPC]
        order = np.argsort(-d_pad, kind="stable")
        rank = np.empty(NP, np.int64)
        rank[order] = np.arange(NP)
        lp_of[k * NPC:(k + 1) * NPC] = rank[:NPC]
        lp_all = np.arange(NP)
        gid_pos = (lp_all // CR) * GCH + k * CR + (lp_all % CR)
        gid_pos_all.append(gid_pos)
        gid_of[k * NPC:(k + 1) * NPC] = gid_pos[rank[:NPC]]
        T1_k[k] = d_pad[order].reshape(NT, P).max(1) + 1
    T1 = T1_k.max(0)                      # L1 slots per bin (incl self)
    OFF1 = np.zeros(NT + 1, np.int64)
    OFF1[1:] = np.cumsum(T1)
    MT1 = int(OFF1[-1])

    # ---- per-core edge lists grouped by dst, split into A/B sides ----
    per_core = []
    cntA = np.zeros((NCORES, NP), np.int64)
    cntF = np.zeros((NCORES, NP), np.int64)
    cntT = np.zeros((NCORES, NP), np.int64)
    for k in range(NCORES):
        m = (dst >= k * NPC) & (dst < (k + 1) * NPC)
        ed = lp_of[dst[m]]
        sg = gid_of[src[m]]
        o = np.argsort(ed, kind="stable")
        ed, sg = ed[o], sg[o]
        per_core.append((ed, sg))
        isA = sg < ALO
        isF = (sg >= ALO) & (sg < AHI)
        cntA[k] = np.bincount(ed[isA], minlength=NP)
        cntF[k] = np.bincount(ed[isF], minlength=NP)
        cntT[k] = np.bincount(ed, minlength=NP)

    # joint per-bin common level c: minimize TA+TB across all cores
    TA = np.zeros(NT, np.int64)
    TB = np.zeros(NT, np.int64)
    CB = np.zeros(NT, np.int64)
    for b in range(NT):
        sl = slice(b * P, (b + 1) * P)
        a = cntA[:, sl].ravel()
        f = cntF[:, sl].ravel()
        t = cntT[:, sl].ravel()
        best = (1 << 40, 0, 0, 0)
        for c in range(0, int(t.max()) + 1):
            Ad = np.clip(c, a, a + f)
            ta, tb = int(Ad.max()), int((t - Ad).max())
            if ta + tb < best[0]:
                best = (ta + tb, ta, tb, c)
        _, ta, tb, c = best
        if b < SELF_A_BINS:
            ta += 1
        else:
            tb += 1
        TA[b], TB[b], CB[b] = ta, tb, c
    OFFA = np.zeros(NT + 1, np.int64)
    OFFA[1:] = np.cumsum(TA)
    OFFB = np.zeros(NT + 1, np.int64)
    OFFB[1:] = np.cumsum(TB)
    MTA, MTB = int(OFFA[-1]), int(OFFB[-1])

    xT = np.zeros((IN_C, N + 1), np.float32)   # col N = zero pad source
    xT[:, :N] = x.T

    cores = []
    for k in range(NCORES):
        ed, sg = per_core[k]
        gid_pos = gid_pos_all[k]
        # invert gid -> source node id for xeT building
        counts = np.bincount(ed, minlength=NP)
        starts = np.zeros(NP + 1, np.int64)
        starts[1:] = np.cumsum(counts)
        src_node = src[(dst >= k * NPC) & (dst < (k + 1) * NPC)][
            np.argsort(lp_of[dst[(dst >= k * NPC) & (dst < (k + 1) * NPC)]],
                       kind="stable")]

        # L1: edge-ordered x columns + mask
        xe_cols = np.full((NP, int(T1.max())), N, np.int64)  # N -> zero col
        own_node = np.full(NP, N, np.int64)
        loc = np.arange(k * NPC, (k + 1) * NPC)
        own_node[lp_of[loc]] = loc
        xe_cols[:, 0] = own_node
        jpos = np.arange(len(ed)) - starts[ed]
        xe_cols[ed, 1 + jpos] = src_node
        mask1 = np.zeros((NP, int(T1.max())), np.float32)
        mask1[:, 0] = (own_node < N)
        mask1[ed, 1 + jpos] = 1.0
        # pad slots of real nodes: already 0; self of pad slots: x=0 but
        # mask=0 would zero z -> keep mask=1 for pad-slot selves (out row
        # discarded anyway, but z=1 avoids 1/0).
        mask1[:, 0] = 1.0
        xeT = np.empty((IN_C, MT1 * P), BF16)
        m1 = np.empty((P, MT1), np.float32)
        for b in range(NT):
            blk = xe_cols[b * P:(b + 1) * P, 0:T1[b]]          # [P, T1b]
            cols = xT[:, blk.T.reshape(-1)]                    # [128, T1b*P]
            xeT[:, OFF1[b] * P:OFF1[b + 1] * P] = cols.astype(BF16)
            m1[:, OFF1[b]:OFF1[b + 1]] = mask1[b * P:(b + 1) * P, 0:T1[b]]

        # L2/L3: A/B side index lists
        idxA = np.empty((16, MTA * 8), np.int16)
        idxB = np.empty((16, MTB * 8), np.int16)
        for b in range(NT):
            sl = slice(b * P, (b + 1) * P)
            a, f, t = cntA[k][sl], cntF[k][sl], cntT[k][sl]
            Ad = np.clip(CB[b], a, a + f)
            flatA = np.full((P, int(TA[b])), AHI, np.int64)
            flatB = np.full((P, int(TB[b])), AHI, np.int64)
            posA = np.zeros(P, np.int64)
            posB = np.zeros(P, np.int64)
            if b < SELF_A_BINS:
                flatA[:, 0] = gid_pos[sl]
                posA += 1
            else:
                flatB[:, 0] = gid_pos[sl] - ALO
                posB += 1
            for p in range(P):
                d = b * P + p
                e_sg = sg[starts[d]:starts[d + 1]]
                sA = e_sg[e_sg < ALO]
                sF = e_sg[(e_sg >= ALO) & (e_sg < AHI)]
                sB = e_sg[e_sg >= AHI]
                nfa = int(Ad[p] - a[p])
                la = np.concatenate([sA, sF[:nfa]])
                lb = np.concatenate([sF[nfa:], sB]) - ALO
                flatA[p, posA[p]:posA[p] + len(la)] = la
                flatB[p, posB[p]:posB[p] + len(lb)] = lb
            idxA[:, OFFA[b] * 8:OFFA[b + 1] * 8] = _wrap16(
                flatA.T.reshape(-1))
            idxB[:, OFFB[b] * 8:OFFB[b + 1] * 8] = _wrap16(
                flatB.T.reshape(-1))

        dpos = np.bincount(ed, minlength=NP).astype(np.float32)
        deginv = np.ascontiguousarray(
            (1.0 / np.maximum(dpos, 1.0)).reshape(NT, P).T)
        cores.append(dict(xeT=np.ascontiguousarray(xeT),
                          mask1=np.ascontiguousarray(m1),
                          idxA=np.ascontiguousarray(idxA),
                          idxB=np.ascontiguousarray(idxB),
                          deginv=deginv.astype(np.float32)))
    cfg = dict(T1=T1, OFF1=OFF1, MT1=MT1, TA=TA, TB=TB, OFFA=OFFA,
               OFFB=OFFB, MTA=MTA, MTB=MTB)
    return cfg, cores, lp_of


def fold_weights(W1, a1s, a1d, b1, W2, a2s, a2d, b2, Wl, bl, Wr, M1, mb1, M2,
                 mb2):
    f = lambda a: np.asarray(a, np.float32)
    W1, a1s, a1d, b1 = f(W1), f(a1s), f(a1d), f(b1)
    W2, a2s, a2d, b2 = f(W2), f(a2s), f(a2d), f(b2)
    Wl, bl, Wr = f(Wl), f(bl), f(Wr)
    M1, mb1, M2, mb2 = f(M1), f(mb1), f(M2), f(mb2)

    def bd(a):
        out = np.zeros((C, HEADS), np.float32)
        for h in range(HEADS):
            out[h * HID:(h + 1) * HID, h] = a[h]
        return out

    def cat(W, as_, ad_):
        out = np.zeros((W.shape[0], COLS), np.float32)
        out[:, 0:C] = W
        out[:, C:C + 3] = W @ bd(as_)
        out[:, 196:199] = W @ bd(ad_)
        return out.astype(BF16)

    padrow = np.zeros((1, COLS), np.float32)
    padrow[0, C:C + 3] = -1000.0
    return dict(
        w1cat=cat(W1, a1s, a1d),
        w2cat=cat(W2, a2s, a2d),
        wl16=np.ascontiguousarray(Wl @ M1 @ M2).astype(BF16),
        wr16=np.ascontiguousarray(Wr @ M1 @ M2).astype(BF16),
        brep1=np.ascontiguousarray(np.tile(b1[None, :], (P, 1))),
        brep2=np.ascontiguousarray(np.tile(b2[None, :], (P, 1))),
        crep=np.ascontiguousarray(
            np.tile((bl @ M1 @ M2 + mb1 @ M2 + mb2)[None, :], (P, 1))),
        padrow=padrow.astype(BF16),
    )


def build_program(cfg, reps=1, phases=(1, 2, 3), ag=True):
    import concourse.bacc as bacc
    import concourse.mybir as mybir
    import concourse.tile as tile
    from concourse.masks import make_identity

    T1, OFF1, MT1 = cfg["T1"], cfg["OFF1"], cfg["MT1"]
    TA, TB, OFFA, OFFB = cfg["TA"], cfg["TB"], cfg["OFFA"], cfg["OFFB"]
    MTA, MTB = cfg["MTA"], cfg["MTB"]
    T1M = int(T1.max())
    TAB = TA + TB
    GROUPS = [(g0, min(g0 + GRP, NT)) for g0 in range(0, NT, GRP)]
    GGM = max(int(TAB[g0:g1].sum()) for g0, g1 in GROUPS)
    GTM = max(T1M, GGM)
    f32 = mybir.dt.float32
    bf16 = mybir.dt.bfloat16
    A = mybir.AluOpType
    ACT = mybir.ActivationFunctionType
    AX = mybir.AxisListType
    i16 = mybir.dt.int16

    nc = bacc.Bacc("TRN2", target_bir_lowering=False, num_devices=NCORES,
                   num_swdge_queues=4)

    xeT = nc.dram_tensor("xeT", [IN_C, MT1 * P], bf16, kind="ExternalInput")
    mask1 = nc.dram_tensor("mask1", [P, MT1], f32, kind="ExternalInput")
    idxA = nc.dram_tensor("idxA", [16, MTA * 8], i16, kind="ExternalInput")
    idxB = nc.dram_tensor("idxB", [16, MTB * 8], i16, kind="ExternalInput")
    deginv = nc.dram_tensor("deginv", [P, NT], f32, kind="ExternalInput")
    w1cat = nc.dram_tensor("w1cat", [IN_C, COLS], bf16, kind="ExternalInput")
    w2cat = nc.dram_tensor("w2cat", [C, COLS], bf16, kind="ExternalInput")
    wl16 = nc.dram_tensor("wl16", [C, OUT_C], bf16, kind="ExternalInput")
    wr16 = nc.dram_tensor("wr16", [C, OUT_C], bf16, kind="ExternalInput")
    brep1 = nc.dram_tensor("brep1", [P, C], f32, kind="ExternalInput")
    brep2 = nc.dram_tensor("brep2", [P, C], f32, kind="ExternalInput")
    crep = nc.dram_tensor("crep", [P, OUT_C], f32, kind="ExternalInput")
    padrow = nc.dram_tensor("padrow", [1, COLS], bf16, kind="ExternalInput")
    out_sh = nc.dram_tensor("out_sh", [NP, OUT_C], f32, kind="ExternalOutput")

    g2loc = nc.dram_tensor("g2loc", [NP, COLS], bf16, kind="Internal")
    yloc = nc.dram_tensor("yloc", [NP, YCOLS], f32, kind="Internal")
    f3r = nc.dram_tensor("f3r", [NP, OUT_C], f32, kind="Internal")
    tabA = nc.dram_tensor("tabA", [TSIZE, COLS], bf16, kind="Internal",
                          addr_space="Local")
    tabB = nc.dram_tensor("tabB", [TSIZE, COLS], bf16, kind="Internal",
                          addr_space="Local")
    yA = nc.dram_tensor("yA", [TSIZE, YCOLS], f32, kind="Internal",
                        addr_space="Local")
    yB = nc.dram_tensor("yB", [TSIZE, YCOLS], f32, kind="Internal",
                        addr_space="Local")

    groups = [list(range(NCORES))]
    qctr = [0]

    def next_q():
        qctr[0] += 1
        return qctr[0] % 4

    with tile.TileContext(nc) as tc:
        import contextlib
        ctx = contextlib.ExitStack()
        with ctx:
            cpool = ctx.enter_context(tc.tile_pool(name="const", bufs=1))
            epool = ctx.enter_context(tc.tile_pool(name="edge", bufs=2))
            spool = ctx.enter_context(tc.tile_pool(name="small", bufs=3))
            accps = ctx.enter_context(
                tc.tile_pool(name="accps", bufs=2, space="PSUM"))
            trps = ctx.enter_context(
                tc.tile_pool(name="trps", bufs=2, space="PSUM"))
            sps = ctx.enter_context(
                tc.tile_pool(name="sps", bufs=2, space="PSUM"))

            ident = cpool.tile([P, P], bf16)
            make_identity(nc, ident[:])
            w1_sb = cpool.tile([IN_C, COLS], bf16)
            nc.sync.dma_start(w1_sb[:], w1cat[:, :])
            w2a_sb = cpool.tile([P, COLS], bf16)
            w2b_sb = cpool.tile([C - P, COLS], bf16)
            nc.sync.dma_start(w2a_sb[:], w2cat[0:P, :])
            nc.sync.dma_start(w2b_sb[:], w2cat[P:C, :])
            wl_a = cpool.tile([P, OUT_C], bf16)
            wl_b = cpool.tile([C - P, OUT_C], bf16)
            wr_a = cpool.tile([P, OUT_C], bf16)
            wr_b = cpool.tile([C - P, OUT_C], bf16)
            nc.sync.dma_start(wl_a[:], wl16[0:P, :])
            nc.sync.dma_start(wl_b[:], wl16[P:C, :])
            nc.sync.dma_start(wr_a[:], wr16[0:P, :])
            nc.sync.dma_start(wr_b[:], wr16[P:C, :])
            b1_sb = cpool.tile([P, C], f32)
            b2_sb = cpool.tile([P, C], f32)
            c_sb = cpool.tile([P, OUT_C], f32)
            nc.sync.dma_start(b1_sb[:], brep1[:, :])
            nc.sync.dma_start(b2_sb[:], brep2[:, :])
            nc.sync.dma_start(c_sb[:], crep[:, :])
            m1_sb = cpool.tile([P, MT1], f32)
            nc.sync.dma_start(m1_sb[:], mask1[:, :])
            dinv_sb = cpool.tile([P, NT], f32)
            nc.sync.dma_start(dinv_sb[:], deginv[:, :])
            ixA_sb = cpool.tile([P, MTA * 8], i16)
            ixB_sb = cpool.tile([P, MTB * 8], i16)
            for g in range(8):
                nc.sync.dma_start(ixA_sb[16 * g:16 * (g + 1), :], idxA[:, :])
                nc.sync.dma_start(ixB_sb[16 * g:16 * (g + 1), :], idxB[:, :])
            pad_sb = cpool.tile([1, COLS], bf16)
            nc.sync.dma_start(pad_sb[:], padrow[:, :])
            zero_y = cpool.tile([1, YCOLS], f32)
            nc.vector.memset(zero_y[:], 0.0)
            nc.sync.dma_start(tabA[AHI:AHI + 1, :], pad_sb[:])
            nc.sync.dma_start(tabB[AHI:AHI + 1, :], pad_sb[:])
            nc.sync.dma_start(yA[AHI:AHI + 1, :], zero_y[:])
            nc.sync.dma_start(yB[AHI:AHI + 1, :], zero_y[:])

            def softmax_agg(G, ranges, self_lo, b_sb, mask_off):
                """w=exp(lrelu(als+ald)) (*mask), scale msgs, reduce, norm
                over one or two disjoint slot ranges of G.
                Returns fsb [P, COLS] bf16 with cols 0:C = relu(out + bias)."""
                sref = G[:, self_lo:self_lo + 1, 196:199]
                reds = []
                for ri, (lo, tt) in enumerate(ranges):
                    att = G[:, lo:lo + tt, C:C + 3]
                    nc.vector.tensor_tensor(
                        out=att, in0=att, in1=sref.broadcast_to([P, tt, 3]),
                        op=A.add)
                    e1 = epool.tile([P, GTM, 3], bf16, tag="e1")
                    e2 = epool.tile([P, GTM, 3], bf16, tag="e2")
                    nc.scalar.activation(e1[:, 0:tt, :], att, ACT.Exp)
                    nc.scalar.activation(e2[:, 0:tt, :], att, ACT.Exp,
                                         scale=0.2)
                    nc.vector.tensor_tensor(out=att, in0=e1[:, 0:tt, :],
                                            in1=e2[:, 0:tt, :], op=A.max)
                    if mask_off is not None:
                        nc.vector.tensor_tensor(
                            out=att, in0=att,
                            in1=m1_sb[:, mask_off:mask_off + tt].unsqueeze(
                                2).broadcast_to([P, tt, 3]),
                            op=A.mult)
                    gh = G[:, lo:lo + tt, 0:C].rearrange(
                        "p t (h c) -> p t h c", h=HEADS)
                    nc.vector.tensor_tensor(
                        out=gh, in0=gh,
                        in1=att.unsqueeze(3).broadcast_to([P, tt, HEADS, HID]),
                        op=A.mult)
                    red = epool.tile([P, 196], f32, tag=f"red{ri}")
                    nc.vector.tensor_reduce(
                        out=red[:], in_=G[:, lo:lo + tt, 0:196].rearrange(
                            "p t c -> p c t"),
                        axis=AX.X, op=A.add)
                    reds.append(red)
                red = reds[0]
                if len(reds) == 2:
                    nc.vector.tensor_tensor(out=red[:], in0=red[:],
                                            in1=reds[1][:], op=A.add)
                zinv = spool.tile([P, 3], f32, tag="zinv")
                nc.vector.reciprocal(zinv[:], red[:, C:C + 3])
                osb = epool.tile([P, C], f32, tag="osb")
                nc.vector.tensor_tensor(
                    out=osb[:].rearrange("p (h c) -> p h c", h=HEADS),
                    in0=red[:, 0:C].rearrange("p (h c) -> p h c", h=HEADS),
                    in1=zinv[:].to_broadcast([P, HEADS, HID]),
                    op=A.mult)
                nc.vector.tensor_tensor(out=osb[:], in0=osb[:], in1=b_sb[:],
                                        op=A.add)
                fsb = epool.tile([P, COLS], bf16, tag="fsb")
                nc.scalar.activation(fsb[:, 0:C], osb[:], ACT.Relu)
                return fsb

            def transpose_blocks(fsb):
                """fsb [P, >=C] bf16 -> two lhsT tiles ([P,128],[64,128])."""
                fts = []
                for k0, kw in ((0, P), (P, C - P)):
                    tp = trps.tile([P, P], bf16, tag="tp")
                    nc.tensor.transpose(out=tp[:kw, :], in_=fsb[:, k0:k0 + kw],
                                        identity=ident[:])
                    ft = epool.tile([P, P], bf16, tag="ft")
                    nc.vector.tensor_copy(ft[:kw, :], tp[:kw, :])
                    fts.append((ft, kw))
                return fts

            for _ in range(reps):
                # ---------------- layer 1 (edge-ordered x, no gather) ------
                for b in range(NT if 1 in phases else 0):
                    TT = int(T1[b])
                    o0 = int(OFF1[b])
                    xe = epool.tile([P, T1M * P], bf16, tag="xe")
                    nc.sync.dma_start(xe[:, 0:TT * P],
                                      xeT[:, o0 * P:(o0 + TT) * P])
                    G = epool.tile([P, GTM, COLS], bf16, tag="G")
                    for j0 in range(0, TT, 2):
                        jn = min(2, TT - j0)
                        ps = accps.tile([P, 2, COLS], f32, tag="acc")
                        for j in range(j0, j0 + jn):
                            nc.tensor.matmul(out=ps[:, j - j0, :],
                                             lhsT=xe[:, j * P:(j + 1) * P],
                                             rhs=w1_sb[:], start=True,
                                             stop=True)
                        nc.vector.tensor_copy(G[:, j0:j0 + jn, :],
                                              ps[:, 0:jn, :])
                    fsb = softmax_agg(G, [(0, TT)], 0, b1_sb, o0)
                    # fused dense-2 -> g2loc
                    ps2 = accps.tile([P, COLS], f32, tag="acc")
                    for bi, ((ft, kw), wt) in enumerate(
                            zip(transpose_blocks(fsb), (w2a_sb, w2b_sb))):
                        nc.tensor.matmul(out=ps2[:], lhsT=ft[:kw, :],
                                         rhs=wt[:], start=(bi == 0),
                                         stop=(bi == 1))
                    g2sb = epool.tile([P, COLS], bf16, tag="g2sb")
                    nc.vector.tensor_copy(g2sb[:], ps2[:])
                    nc.sync.dma_start(g2loc[b * P:(b + 1) * P, :], g2sb[:])
                    if ag and b % NCH == NCH - 1:
                        cc = b // NCH
                        if cc <= 3:
                            nc.gpsimd.collective_compute(
                                "AllGather", A.bypass, replica_groups=groups,
                                ins=[g2loc[cc * CR:(cc + 1) * CR, :]],
                                outs=[tabA[cc * GCH:(cc + 1) * GCH, :]])
                        if cc >= 3:
                            nc.gpsimd.collective_compute(
                                "AllGather", A.bypass, replica_groups=groups,
                                ins=[g2loc[cc * CR:(cc + 1) * CR, :]],
                                outs=[tabB[(cc - 3) * GCH:(cc - 2) * GCH, :]])

                # ---------------- layer 2 (grouped gathers) ----------------
                for g0, g1 in (GROUPS if 2 in phases else []):
                    tAs = [int(TA[b]) for b in range(g0, g1)]
                    tBs = [int(TB[b]) for b in range(g0, g1)]
                    sAg, sBg = sum(tAs), sum(tBs)
                    G = epool.tile([P, GTM, COLS], bf16, tag="G")
                    nc.gpsimd.dma_gather(
                        G[:, 0:sAg, :], tabA[:, :],
                        ixA_sb[:, OFFA[g0] * 8:OFFA[g1] * 8],
                        sAg * P, sAg * P, COLS, single_packet=False,
                        queue_num=next_q())
                    nc.gpsimd.dma_gather(
                        G[:, sAg:sAg + sBg, :], tabB[:, :],
                        ixB_sb[:, OFFB[g0] * 8:OFFB[g1] * 8],
                        sBg * P, sBg * P, COLS, single_packet=False,
                        queue_num=next_q())
                    aoff, boff = 0, sAg
                    for bi, b in enumerate(range(g0, g1)):
                        tA, tB = tAs[bi], tBs[bi]
                        self_lo = aoff if b < SELF_A_BINS else boff
                        fsb = softmax_agg(
                            G, [(aoff, tA), (boff, tB)], self_lo, b2_sb, None)
                        # epilogue: y = f3@Wl', r = f3@Wr'
                        psy = sps.tile([P, OUT_C], f32, tag="psy")
                        psr = sps.tile([P, OUT_C], f32, tag="psr")
                        for ki, ((ft, kw), wlt, wrt) in enumerate(
                                zip(transpose_blocks(fsb), (wl_a, wl_b),
                                    (wr_a, wr_b))):
                            nc.tensor.matmul(out=psy[:], lhsT=ft[:kw, :],
                                             rhs=wlt[:], start=(ki == 0),
                                             stop=(ki == 1))
                            nc.tensor.matmul(out=psr[:], lhsT=ft[:kw, :],
                                             rhs=wrt[:], start=(ki == 0),
                                             stop=(ki == 1))
                        ysb = epool.tile([P, YCOLS], f32, tag="ysb")
                        nc.vector.memset(ysb[:, OUT_C:YCOLS], 0.0)
                        nc.vector.tensor_copy(ysb[:, 0:OUT_C], psy[:])
                        nc.sync.dma_start(yloc[b * P:(b + 1) * P, :], ysb[:])
                        rsb = spool.tile([P, OUT_C], f32, tag="rsb")
                        nc.vector.tensor_copy(rsb[:], psr[:])
                        nc.sync.dma_start(f3r[b * P:(b + 1) * P, :], rsb[:])
                        if ag and b % NCH == NCH - 1:
                            cc = b // NCH
                            if cc <= 3:
                                nc.gpsimd.collective_compute(
                                    "AllGather", A.bypass,
                                    replica_groups=groups,
                                    ins=[yloc[cc * CR:(cc + 1) * CR, :]],
                                    outs=[yA[cc * GCH:(cc + 1) * GCH, :]])
                            if cc >= 3:
                                nc.gpsimd.collective_compute(
                                    "AllGather", A.bypass,
                                    replica_groups=groups,
                                    ins=[yloc[cc * CR:(cc + 1) * CR, :]],
                                    outs=[yB[(cc - 3) * GCH:
                                             (cc - 2) * GCH, :]])
                        aoff += tA
                        boff += tB

                # ------- SAGE + MLP head (same slots as L2, minus self) ----
                for g0, g1 in (GROUPS if 3 in phases else []):
                    tAs = [int(TA[b]) for b in range(g0, g1)]
                    tBs = [int(TB[b]) for b in range(g0, g1)]
                    sAg, sBg = sum(tAs), sum(tBs)
                    Gy = epool.tile([P, GTM, YCOLS], f32, tag="Gy")
                    nc.gpsimd.dma_gather(
                        Gy[:, 0:sAg, :], yA[:, :],
                        ixA_sb[:, OFFA[g0] * 8:OFFA[g1] * 8],
                        sAg * P, sAg * P, YCOLS, single_packet=False,
                        queue_num=next_q())
                    nc.gpsimd.dma_gather(
                        Gy[:, sAg:sAg + sBg, :], yB[:, :],
                        ixB_sb[:, OFFB[g0] * 8:OFFB[g1] * 8],
                        sBg * P, sBg * P, YCOLS, single_packet=False,
                        queue_num=next_q())
                    aoff, boff = 0, sAg
                    for bi, b in enumerate(range(g0, g1)):
                        tA, tB = tAs[bi], tBs[bi]
                        red = spool.tile([P, OUT_C], f32, tag="redy")
                        nc.vector.tensor_reduce(
                            out=red[:],
                            in_=Gy[:, aoff:aoff + tA, 0:OUT_C].rearrange(
                                "p t c -> p c t"),
                            axis=AX.X, op=A.add)
                        red2 = spool.tile([P, OUT_C], f32, tag="redy2")
                        nc.vector.tensor_reduce(
                            out=red2[:],
                            in_=Gy[:, boff:boff + tB, 0:OUT_C].rearrange(
                                "p t c -> p c t"),
                            axis=AX.X, op=A.add)
                        nc.vector.tensor_tensor(out=red[:], in0=red[:],
                                                in1=red2[:], op=A.add)
                        # subtract the self-loop y (sum included it)
                        yown = spool.tile([P, OUT_C], f32, tag="yown")
                        nc.sync.dma_start(yown[:],
                                          yloc[b * P:(b + 1) * P, 0:OUT_C])
                        nc.vector.tensor_tensor(out=red[:], in0=red[:],
                                                in1=yown[:], op=A.subtract)
                        agg = spool.tile([P, OUT_C], f32, tag="agg")
                        nc.vector.tensor_scalar(
                            out=agg[:], in0=red[:],
                            scalar1=dinv_sb[:, b:b + 1],
                            scalar2=None, op0=A.mult)
                        f3rt = spool.tile([P, OUT_C], f32, tag="f3rt")
                        nc.sync.dma_start(f3rt[:], f3r[b * P:(b + 1) * P, :])
                        nc.vector.tensor_tensor(out=agg[:], in0=agg[:],
                                                in1=f3rt[:], op=A.add)
                        nc.vector.tensor_tensor(out=agg[:], in0=agg[:],
                                                in1=c_sb[:], op=A.add)
                        sig = spool.tile([P, OUT_C], f32, tag="sig")
                        nc.scalar.activation(sig[:], agg[:], ACT.Sigmoid)
                        nc.sync.dma_start(out_sh[b * P:(b + 1) * P, :],
                                          sig[:])
                        aoff += tA
                        boff += tB

    nc.compile()
    return nc


LAST_RESULTS = None


def kernel(**inputs):
    global LAST_RESULTS
    import os
    x = np.asarray(inputs["x"], np.float32)
    edge_index = np.asarray(inputs["edge_index"])
    cfg, cores, lp_of = preprocess(x, edge_index)
    wts = fold_weights(
        inputs["W1"], inputs["a1s"], inputs["a1d"], inputs["b1"],
        inputs["W2"], inputs["a2s"], inputs["a2d"], inputs["b2"],
        inputs["Wl"], inputs["bl"], inputs["Wr"],
        inputs["M1"], inputs["mb1"], inputs["M2"], inputs["mb2"])
    nc = build_program(cfg)
    in_maps = [dict(core, **wts) for core in cores]

    from concourse import bass_utils
    res = bass_utils.run_bass_kernel_spmd(
        nc, in_maps, core_ids=list(range(NCORES)),
        trace=bool(int(os.environ.get("GAT_TRACE", "0"))))
    LAST_RESULTS = res
    out = np.zeros((N, OUT_C), np.float32)
    for k in range(NCORES):
        o = res.results[k]["out_sh"]
        lo, hi = k * NPC, (k + 1) * NPC
        out[lo:hi] = o[lp_of[lo:hi]]
    return out
